# revision 3
# baseline (speedup 1.0000x reference)
"""Trainium2 Bass kernel for nn_BSplineKANLayer (B-spline KAN layer).

Math summary
------------
reference computes, for x:(B=16384, D=512):
    grid    = t0 + cumsum(softplus(grid_steps_log))        (uniform: t_j = t0 + j*h)
    bases   = de-Boor cubic B-spline basis  (B, D, 8)
    y       = tanh( bases_flat @ coeffs.T/1 + silu(x) @ W.T + res*x )

For the uniform grid the de-Boor recursion collapses to the cardinal cubic
B-spline:  bases[b,i,j] = N3(s - j),  s = (x - t0)/h,  and by symmetry of N3
    6*N3(t) = relu(2 - |t-2|)^3 - 4*relu(1 - |t-2|)^3
which is bounded, exactly zero outside support, and cancellation-free.

Device pipeline per basis K-tile (j, i-block), all fp32 until the final cast:
    vp  = (s*G - G*(j+2)) abs_max 0          -- one stock DVE tensor_scalar
    za  = Square(-D/G * vp + 2*D)            -- one ACT op (D=G^-1/2, so a*za below is the cube)
    bas = relu(2G - vp)*za - relu((2G-vp)-G)^3   -- one 8-stage custom DVE op -> fp16
(G = 4^(1/3) so the second relu's cube carries the factor 4.)

The two GEMMs run in fp16 on TensorE with fp32 PSUM accumulation, contracting
over K = (j,i) [4096 rows] plus 512 silu rows, output transposed: yt (D, shard).

Sharding: data-parallel over batch across 8 cores; weights replicated.
Host does layout only: x -> xT shards, coeffs -> K-tiled fp16 weight image,
final yt -> y transpose.
"""

from contextlib import ExitStack

import numpy as np
import concourse.bass as bass
import concourse.bacc as bacc
import concourse.tile as tile
from concourse import mybir
from concourse.bass_utils import run_bass_kernel_spmd

# ---- problem constants (hardcoded per spec) --------------------------------
B = 16384
D = 512
NC8 = 8                      # n_coeffs per input dim
NCORES = 8
SHARD = B // NCORES          # 2048 batch rows per core
SLAB = 1024                  # batch columns processed per accumulation pass
NSLAB = SHARD // SLAB        # 2
KSP = (D * NC8) // 128       # 32 spline K-tiles of 128
KTOT = KSP + D // 128        # 36 (incl. 4 silu K-tiles)
OB = D // 128                # 4 output blocks
IBK = D // 128               # 4 input blocks
GAMMA = float(4.0 ** (1.0 / 3.0))
DELTA = float(GAMMA ** -0.5)

F32 = mybir.dt.float32
F16 = mybir.dt.float16

# ---- custom DVE op registration (runtime, idempotent) ----------------------
_OPS_REGISTERED = {}


def _register_dve_ops():
    """Register the three custom DVE ops used by the kernel (idempotent).

    BSPL_CUBE8: out = relu(C0-in0)*in1 - relu((C0-in0)-C1)^3        (vp input)
    HAT2_ANT:   out = C1 - |in0 - C0|                               (w2 = G*(2-v))
    BSPL_CUBE7: out = relu(in0)*in1 - relu(in0-C1)^3                (w2 input)
    """
    global _OPS_REGISTERED
    if _OPS_REGISTERED:
        return _OPS_REGISTERED
    import concourse.dve_ops as dve_ops
    from concourse.dve_ops import DveOp
    from concourse.dve_spec import (
        Spec, Src0, Src1, C0, C1, relu, sq, lower, _has_src1, Bin, AluOp,
    )
    from concourse.dve_uop import DveOpSpec

    def cube8_ref(in0, in1, s0, s1, imm2):
        w2 = np.float32(s0) - in0.astype(np.float32)
        a = np.maximum(w2, np.float32(0))
        bb = np.maximum(w2 - np.float32(s1), np.float32(0))
        return (a * in1.astype(np.float32) - bb * bb * bb).astype(np.float32)

    w2 = C0 - Src0
    cube8_body = relu(w2) * Src1 - sq(relu(w2 - C1)) * relu(w2 - C1)

    def hat2_ref(in0, in1, s0, s1, imm2):
        return (np.float32(s1) - np.abs(in0.astype(np.float32) - np.float32(s0))
                ).astype(np.float32)

    hat2_body = C1 - Bin(AluOp.ABSOLUTE_DIFF, Src0, C0)

    def cube7_ref(in0, in1, s0, s1, imm2):
        w2 = in0.astype(np.float32)
        a = np.maximum(w2, np.float32(0))
        bb = np.maximum(w2 - np.float32(s1), np.float32(0))
        return (a * in1.astype(np.float32) - bb * bb * bb).astype(np.float32)

    cube7_body = relu(Src0) * Src1 - sq(relu(Src0 - C1)) * relu(Src0 - C1)

    def absd_ref(in0, in1, s0, s1, imm2):
        return np.abs(in0.astype(np.float32) - np.float32(s0)).astype(np.float32)

    absd_body = Bin(AluOp.ABSOLUTE_DIFF, Src0, C0)

    specs = [
        ("BSPLINE_CUBE_ANT", cube8_body, cube8_ref),
        ("HAT2_ANT", hat2_body, hat2_ref),
        ("BSPLINE_CUBE7_ANT", cube7_body, cube7_ref),
        ("ABSD_ANT", absd_body, absd_ref),
    ]
    existing = {op.name: op for op in dve_ops.OPS}
    for name, body, ref in specs:
        if name in existing:
            _OPS_REGISTERED[name] = existing[name]
            continue
        op = DveOp(name, Spec(body=body, reference=ref), subdim=False, uops_sha={})
        row = max(dve_ops._SUB_OPCODE_FOR_NAME.values()) + 1
        assert row < 0x20
        dve_ops._SUB_OPCODE_FOR_NAME[name] = row
        for ver in ("v3", "v4"):
            try:
                uops = lower(op.spec, ver=ver)
                tmp = DveOpSpec(name=name, opcode=row, uops=uops,
                                rd1_en=_has_src1(op.spec))
                op.uops_sha[ver] = tmp.sha(ver)
            except Exception:
                pass
        dve_ops.OPS.append(op)
        dve_ops.CUSTOM_DVE_SPECS[name] = op.spec
        _OPS_REGISTERED[name] = op
    return _OPS_REGISTERED


# ---- device kernel ---------------------------------------------------------

def _build_nc(t0: float, h: float, res: float, tune: dict | None = None) -> bass.Bass:
    tune = dict(tune or {})
    ops = _register_dve_ops()
    CUBE8, ABSD = ops["BSPLINE_CUBE_ANT"], ops["ABSD_ANT"]
    nc = bacc.Bacc(
        "TRN2", target_bir_lowering=False, debug=False, num_devices=NCORES
    )
    xt = nc.declare_dram_parameter("xt", [D, SHARD], F32, isOutput=False)
    ct = nc.declare_dram_parameter("ct", [128, KTOT * 4 * 128], F16, isOutput=False)
    yt = nc.declare_dram_parameter("yt", [D, SHARD], F32, isOutput=True)

    sg_scale = GAMMA / h          # sg = x*sg_scale + sg_bias = GAMMA*(x-t0)/h
    sg_bias = -GAMMA * t0 / h
    AF = mybir.ActivationFunctionType
    ALU = mybir.AluOpType

    with tile.TileContext(nc) as tc, ExitStack() as ctx:
        const_pool = ctx.enter_context(tc.tile_pool(name="constp", bufs=1))
        ct_pool = ctx.enter_context(tc.tile_pool(name="ctp", bufs=1))
        xt_pool = ctx.enter_context(tc.tile_pool(name="xtp", bufs=tune.get("xt", 2)))
        sg_pool = ctx.enter_context(tc.tile_pool(name="sgp", bufs=tune.get("sg", 2)))
        sx_pool = ctx.enter_context(tc.tile_pool(name="sxp", bufs=tune.get("sx", 2)))
        vp_pool = ctx.enter_context(tc.tile_pool(name="vpp", bufs=tune.get("vp", 3)))
        za_pool = ctx.enter_context(tc.tile_pool(name="zap", bufs=tune.get("za", 3)))
        bas_pool = ctx.enter_context(tc.tile_pool(name="basp", bufs=tune.get("bas", 3)))
        out_pool = ctx.enter_context(tc.tile_pool(name="outp", bufs=tune.get("out", 4)))
        ps_pool = ctx.enter_context(
            tc.tile_pool(name="psp", bufs=1, space=bass.MemorySpace.PSUM)
        )

        # per-partition constants for ACT bias slots
        za_bias = const_pool.tile([128, 1], F32, tag="za_bias", name="za_bias")
        nc.gpsimd.memset(za_bias[:], float(2.0 * DELTA))
        vp_bias = const_pool.tile([128, NC8], F32, tag="vp_bias", name="vp_bias")
        for j in range(NC8):
            nc.gpsimd.memset(vp_bias[:, j:j + 1], float(-GAMMA * (j + 2.0)))

        ct_t = ct_pool.tile([128, KTOT * 4 * 128], F16)

        reps = int(tune.get("reps", 1))

        if tune.get("warmup", 1):
            wu_pool = ctx.enter_context(tc.tile_pool(name="wup", bufs=1))
            wu = wu_pool.tile([128, 512], F16, name="wu")
            nc.gpsimd.memset(wu[:], 0.0)
            wu_ps = ps_pool.tile([128, 512], F32, tag="ps0_0", name="wu_ps")
            for i in range(int(tune.get("warmup_n", 40))):
                nc.tensor.matmul(wu_ps[:], wu[:, 0:128], wu[:],
                                 start=True, stop=True)

        def _full_body():
            ct_loaded = [False]

            def _load_ct():
                if ct_loaded[0]:
                    return
                ct_loaded[0] = True
                if tune.get("ct_memset"):
                    nc.gpsimd.memset(ct_t[:], 0.25)
                    return
                base_sz = 4 * 4 * 128          # base-gemm tiles (k=0..3)
                nc.sync.dma_start(ct_t[:, 0:base_sz], ct[:, 0:base_sz])
                rest = KTOT * 4 * 128 - base_sz
                csz = rest // 4
                for c in range(4):
                    nc.sync.dma_start(
                        ct_t[:, base_sz + c * csz: base_sz + (c + 1) * csz],
                        ct[:, base_sz + c * csz: base_sz + (c + 1) * csz],
                    )

            for slab in range(int(tune.get("nslab", NSLAB))):
                xts, sgs, sxs = [], [], []
                for ib in range(IBK):
                    xt_t = xt_pool.tile([128, SLAB], F32, tag=f"xt{ib}", name=f"xt{ib}")
                    nc.sync.dma_start(
                        xt_t[:],
                        xt[ib * 128:(ib + 1) * 128, bass.ts(slab, SLAB)],
                    )
                    _load_ct()
                    sg_t = sg_pool.tile([128, SLAB], F32, tag=f"sg{ib}", name=f"sg{ib}")
                    nc.vector.tensor_scalar(
                        sg_t[:], xt_t[:], float(sg_scale), float(sg_bias),
                        ALU.mult, ALU.add,
                    )
                    sx_t = sx_pool.tile([128, SLAB], F16, tag=f"sx{ib}", name=f"sx{ib}")
                    nc.scalar.activation(sx_t[:], xt_t[:], AF.Silu)
                    xts.append(xt_t)
                    sgs.append(sg_t)
                    sxs.append(sx_t)

                psums = [
                    [
                        ps_pool.tile([128, 512], F32, tag=f"ps{o}_{n2}", name=f"ps{o}_{n2}")
                        for n2 in range(SLAB // 512)
                    ]
                    for o in range(OB)
                ]

                for kb in range(IBK):
                    for o in range(OB):
                        for n2 in range(SLAB // 512):
                            nc.tensor.matmul(
                                psums[o][n2][:],
                                ct_t[:, bass.ts(kb * 4 + o, 128)],
                                sxs[kb][:, bass.ts(n2, 512)],
                                start=(kb == 0),
                                stop=(tune.get("skip_mm", False) and kb == IBK - 1),
                            )

                for j0 in range(0, NC8, 2):
                    for ib in range(IBK):
                        if tune.get("skip_elem"):
                            for dj in range(2):
                                k = 4 + (j0 + dj) * IBK + ib
                                for o in range(OB):
                                    for n2 in range(SLAB // 512):
                                        nc.tensor.matmul(
                                            psums[o][n2][:],
                                            ct_t[:, bass.ts(k * 4 + o, 128)],
                                            sxs[ib][:, bass.ts(n2, 512)],
                                            start=False,
                                            stop=(j0 == NC8 - 2 and dj == 1
                                                  and ib == IBK - 1),
                                        )
                            continue
                        vp2 = vp_pool.tile([128, 2 * SLAB], F32, name="vp2")
                        # half 0 on ScalarE (Abs), half 1 on VectorE (ABSOLUTE_DIFF)
                        nc.scalar.activation(
                            vp2[:, 0:SLAB], sgs[ib][:], AF.Abs,
                            bias=vp_bias[:, j0:j0 + 1], scale=1.0,
                        )
                        nc.vector._custom_dve(
                            ABSD, out=vp2[:, SLAB:2 * SLAB], in0=sgs[ib][:],
                            s0=float(GAMMA * (j0 + 3.0)),
                        )
                        za2 = za_pool.tile([128, 2 * SLAB], F32, name="za2")
                        nc.scalar.activation(
                            za2[:], vp2[:], AF.Square,
                            bias=za_bias[:], scale=float(-DELTA / GAMMA),
                        )
                        bas2 = bas_pool.tile([128, 2 * SLAB], F16, name="bas2")
                        nc.vector._custom_dve(
                            CUBE8, out=bas2[:], in0=vp2[:], in1=za2[:],
                            s0=float(2.0 * GAMMA), s1=float(GAMMA),
                        )
                        if not tune.get("skip_mm"):
                            for dj in range(2):
                                k = 4 + (j0 + dj) * IBK + ib
                                for o in range(OB):
                                    for n2 in range(SLAB // 512):
                                        nc.tensor.matmul(
                                            psums[o][n2][:],
                                            ct_t[:, bass.ts(k * 4 + o, 128)],
                                            bas2[:, dj * SLAB + n2 * 512:
                                                 dj * SLAB + (n2 + 1) * 512],
                                            start=False, stop=(j0 == NC8 - 2 and dj == 1
                                                               and ib == IBK - 1),
                                        )
                for o in range(OB):
                    for n2 in range(SLAB // 512):
                        if tune.get("skip_out"):
                            dump = out_pool.tile([128, 1], F32, tag="dump", name="dump")
                            nc.vector.tensor_copy(dump[:], psums[o][n2][:, 0:1])
                            continue
                        out_t = out_pool.tile([128, 512], F32, name="out_t")
                        if res != 0.0:
                            tmp = out_pool.tile([128, 512], F32, tag="tmp", name="tmp")
                            nc.vector.affine_then_add(
                                tmp[:],
                                xts[o][:, bass.ts(n2, 512)],
                                psums[o][n2][:],
                                scale=float(res), bias=0.0,
                            )
                            nc.scalar.activation(out_t[:], tmp[:], AF.Tanh)
                        else:
                            nc.scalar.activation(out_t[:], psums[o][n2][:], AF.Tanh)
                        nc.sync.dma_start(
                            yt[o * 128:(o + 1) * 128,
                               slab * SLAB + n2 * 512: slab * SLAB + (n2 + 1) * 512],
                            out_t[:],
                        )

        if reps > 1:
            with tc.For_i(0, reps, 1):
                _full_body()
        else:
            _full_body()
    nc.compile()
    return nc


_NC_CACHE: dict = {}
_TUNE: dict = {}     # bench-only override; empty for the graded path


def _get_nc(t0: float, h: float, res: float, tune: dict | None = None) -> bass.Bass:
    key = (round(t0, 9), round(h, 9), round(res, 9), tuple(sorted((tune or {}).items())))
    if key not in _NC_CACHE:
        _NC_CACHE[key] = _build_nc(t0, h, res, tune)
    return _NC_CACHE[key]


# ---- host wrapper ----------------------------------------------------------

def _grid_scalars(grid_steps_log, grid_start):
    gsl = np.asarray(grid_steps_log, np.float64)
    steps = np.logaddexp(gsl, 0.0)  # softplus, matches jax.nn.softplus
    g0 = np.asarray(grid_start, np.float64)
    assert np.ptp(steps) < 1e-6 and np.ptp(g0) < 1e-6, (
        "kernel assumes a uniform B-spline grid (shared across dims)"
    )
    return float(g0.reshape(-1)[0]), float(steps.reshape(-1)[0])


def kernel(x, coeffs, base_weight, grid_steps_log, grid_start, res_scale):
    x = np.ascontiguousarray(np.asarray(x, np.float32))
    coeffs = np.asarray(coeffs, np.float32)
    base_weight = np.asarray(base_weight, np.float32)
    assert x.shape == (B, D) and coeffs.shape == (D, D * NC8)

    t0, h = _grid_scalars(grid_steps_log, grid_start)
    res = float(np.asarray(res_scale).reshape(-1)[0])

    # weight image: base rows first, then spline rows (j-major)
    L = np.empty((KTOT * 128, D), np.float32)
    L[:D] = base_weight.T
    L[D:] = coeffs.reshape(D, D, NC8).transpose(2, 1, 0).reshape(D * NC8, D)
    Lb = L.astype(np.float16)
    # ct[p, (k*4+o)*128 + m] = L[k*128+p, o*128+m]
    ct_sb = np.ascontiguousarray(
        Lb.reshape(KTOT, 128, OB, 128).transpose(1, 0, 2, 3).reshape(128, KTOT * 4 * 128)
    )

    nc = _get_nc(t0, h, res, _TUNE or None)
    xT = x.T  # (D, B) view
    in_maps = [
        {
            "xt": np.ascontiguousarray(xT[:, c * SHARD:(c + 1) * SHARD]),
            "ct": ct_sb,
        }
        for c in range(NCORES)
    ]
    r = run_bass_kernel_spmd(nc, in_maps, list(range(NCORES)))

    y = np.empty((B, D), np.float32)
    for c in range(NCORES):
        y[c * SHARD:(c + 1) * SHARD] = r.results[c]["yt"].T
    return y

